# revision 1
# baseline (speedup 1.0000x reference)
"""Trainium2 Bass kernel for relative-position multi-head attention.

Problem shape (hardcoded): B=2, T=1024, CH=1024, HEADS=16, KC=64, WIN=4.
Sharding: tensor-parallel over heads across 8 cores (2 heads/core),
AllGather of head outputs, per-core column shard of the output projection.

Key observation: with T=1024 and window 4, the relative-position logits
(_rel_to_abs) and weights (_abs_to_rel) only touch the 9 diagonals
|j - i| <= 4 of the [T, T] score matrix.  Both the band-add (scores) and
the band-gather (rel_v epilogue) are routed through a small DRAM staging
buffer with a "diagonal-compact" layout: value for (j, i) at flat
(j+4)*144 + (i-j+4).  Rectangular SBUF windows [128, 136] of the [j, i]
plane map to [[143, 128], [1, 136]] patterns there (partition-outer,
contiguous inner runs), and unused slots m in [9, 144) are zeroed once so
out-of-band window cells read zeros.
"""

import sys

sys.path.insert(0, "/opt/trn_rl_repo")

import math
import numpy as np

import concourse.bass as bass
import concourse.tile as tile
from concourse import mybir
from concourse import bacc
from concourse.bass_utils import run_bass_kernel_spmd
from concourse.masks import make_identity

# ---------------------------------------------------------------- constants
B, T, CH, HEADS, KC, WIN = 2, 1024, 1024, 16, 64, 4
NCORES = 8
HPC = HEADS // NCORES          # heads per core = 2
DPC = HPC * KC                 # channels per core = 128
NI = B * T                     # 2048 flattened (b, t)
M9 = 2 * WIN + 1               # 9 diagonals
WSLOT = 144                    # diag-compact row stride (> 135 garbage range)
WBND = 136                     # band window width (i in [j0-4, j0+132))
EXPW = 8 * 1024 + 8            # per-unit exp(S) tensor width (+8 pad cols)
F32 = mybir.dt.float32
F32R = mybir.dt.float32r
AF = mybir.ActivationFunctionType

_CACHE = {}


# ---------------------------------------------------------------- program
def build_program():
    nc = bacc.Bacc("TRN2", target_bir_lowering=False, debug=False,
                   num_devices=NCORES)

    xT = nc.dram_tensor("xT", [CH, NI], F32R, kind="ExternalInput")
    cT = nc.dram_tensor("cT", [CH, NI], F32R, kind="ExternalInput")
    wq = nc.dram_tensor("wq", [CH, DPC], F32R, kind="ExternalInput")
    wk = nc.dram_tensor("wk", [CH, DPC], F32R, kind="ExternalInput")
    wv = nc.dram_tensor("wv", [CH, DPC], F32R, kind="ExternalInput")
    wo = nc.dram_tensor("wo", [CH, DPC], F32R, kind="ExternalInput")
    bq = nc.dram_tensor("bq", [DPC, 1], F32, kind="ExternalInput")
    bk = nc.dram_tensor("bk", [DPC, 1], F32, kind="ExternalInput")
    bv = nc.dram_tensor("bv", [DPC, 1], F32, kind="ExternalInput")
    bo = nc.dram_tensor("bo", [DPC, 1], F32, kind="ExternalInput")
    erk = nc.dram_tensor("erk", [DPC, M9], F32R, kind="ExternalInput")
    erv = nc.dram_tensor("erv", [M9, KC + 1], F32R, kind="ExternalInput")
    zros = nc.dram_tensor("zros", [128, 1164], F32, kind="ExternalInput")
    one8 = nc.dram_tensor("one8", [128, 8], F32R, kind="ExternalInput")
    outT = nc.dram_tensor("outT", [DPC, NI], F32, kind="ExternalOutput")

    with tile.TileContext(nc) as tc:
        with (
            tc.tile_pool(name="const", bufs=1) as cpool,
            tc.tile_pool(name="persist", bufs=1) as ppool,
            tc.tile_pool(name="dram", bufs=1, space="DRAM") as dpool,
        ):
            # ---------------- constants / weights to SBUF (1 DMA each)
            wsb = {}
            for nm, src in (("wq", wq), ("wk", wk), ("wv", wv), ("wo", wo)):
                t_ = cpool.tile([128, 8 * DPC], F32R, name=f"{nm}_sb")
                # chunk d8 at free cols [128*d8, 128*(d8+1)): src row 128*d8+p
                sap = bass.AP(src, 0, [[DPC, 128], [DPC * 128, 8], [1, DPC]])
                nc.scalar.dma_start(
                    t_[:].rearrange("p (c8 c) -> p c8 c", c=DPC), sap)
                wsb[nm] = t_

            def wtile(nm, d8):
                return wsb[nm][:, DPC * d8:DPC * (d8 + 1)]

            bq_sb = cpool.tile([DPC, 1], F32, name="bq_sb")
            bk_sb = cpool.tile([DPC, 1], F32, name="bk_sb")
            bv_sb = cpool.tile([DPC, 1], F32, name="bv_sb")
            bo_sb = cpool.tile([DPC, 1], F32, name="bo_sb")
            for t_, src in ((bq_sb, bq), (bk_sb, bk), (bv_sb, bv), (bo_sb, bo)):
                nc.scalar.dma_start(t_[:], src[:])
            erk_sb = cpool.tile([DPC, M9], F32R, name="erk_sb")
            nc.scalar.dma_start(erk_sb[:], erk[:])
            erv_sb = cpool.tile([M9, KC + 1], F32R, name="erv_sb")
            nc.scalar.dma_start(erv_sb[:], erv[:])
            ident = cpool.tile([128, 128], F32, name="ident")
            make_identity(nc, ident[:])

            # persistent activations
            qT_sb = ppool.tile([DPC, NI], F32R, name="qT_sb")
            kT_sb = ppool.tile([DPC, NI], F32R, name="kT_sb")
            vT_sb = ppool.tile([DPC, NI], F32, name="vT_sb")
            OT_sb = ppool.tile([DPC, NI], F32R, name="OT_sb")
            # v in [j, d] layout + ones column, per unit (b, h): [128, 8*65]
            vju = [ppool.tile([128, 8 * (KC + 1)], F32R, name=f"vju{u}")
                   for u in range(4)]

            # DRAM staging for the diagonal band (double buffered by parity)
            rd_d = [dpool.tile([128 * 1164], F32, name=f"rd{p}")
                    for p in range(2)]
            gd_d = [dpool.tile([(T + 8) * WSLOT], F32R, name=f"gd{p}")
                    for p in range(2)]
            ag_in = dpool.tile([DPC, NI], F32R, name="ag_in")
            ag_out = dpool.tile([NCORES * DPC, NI], F32R, name="ag_out",
                                addr_space="Shared")

            # zero the Rd buffers once (slots >= 9 and j-header/tail stay 0)
            for p in range(2):
                flat = rd_d[p][:].rearrange("(r c) -> r c", c=1164)
                nc.sync.dma_start(flat[:], zros[:])
                # Gd: only the j-invalid edge rows must be zero (read as
                # the clipped rel_w corners); windows never write them.
                flatg = gd_d[p][:].rearrange("(r c) -> r c", c=WSLOT)
                nc.sync.dma_start(flatg[0:4, :],
                                  zros[:4, :WSLOT].bitcast(F32R))
                nc.sync.dma_start(flatg[T + 4:T + 8, :],
                                  zros[:4, :WSLOT].bitcast(F32R))
            # ones columns of vju (one strided DMA per unit)
            for u in range(4):
                dst = bass.AP(vju[u].tensor, KC,
                              [[8 * (KC + 1), 128], [KC + 1, 8]])
                nc.scalar.dma_start(dst, one8[:])

            # ---------------- phase A: QKV projections (transposed layouts)
            with (
                tc.tile_pool(name="xin", bufs=12) as xpool,
                tc.tile_pool(name="qkvps", bufs=1, space="PSUM") as qkvps,
                tc.tile_pool(name="tps", bufs=2, space="PSUM") as tpps,
            ):
                # q: load all 8 row-blocks of xT, 4 open accumulation groups
                xts = []
                for d8 in range(8):
                    t_ = xpool.tile([128, NI], F32R, tag="xt")
                    nc.gpsimd.dma_start(t_[:], xT[d8 * 128:(d8 + 1) * 128, :])
                    xts.append(t_)
                qps = [qkvps.tile([DPC, 512], F32, tag=f"proj{it}",
                                  name=f"qp{it}") for it in range(4)]
                for d8 in range(8):
                    for it in range(4):
                        nc.tensor.matmul(
                            qps[it][:], wtile("wq", d8),
                            xts[d8][:, it * 512:(it + 1) * 512],
                            start=(d8 == 0), stop=(d8 == 7))
                for it in range(4):
                    nc.vector.tensor_scalar_add(
                        qT_sb[:, it * 512:(it + 1) * 512], qps[it][:], bq_sb[:])

                cts = []
                for d8 in range(8):
                    t_ = xpool.tile([128, NI], F32R, tag="xt")
                    nc.gpsimd.dma_start(t_[:], cT[d8 * 128:(d8 + 1) * 128, :])
                    cts.append(t_)
                kps = [qkvps.tile([DPC, 512], F32, tag=f"proj{it}",
                                  name=f"kp{it}") for it in range(4)]
                for d8 in range(8):
                    for it in range(4):
                        nc.tensor.matmul(
                            kps[it][:], wtile("wk", d8),
                            cts[d8][:, it * 512:(it + 1) * 512],
                            start=(d8 == 0), stop=(d8 == 7))
                for it in range(4):
                    nc.scalar.activation(kT_sb[:, it * 512:(it + 1) * 512],
                                         kps[it][:], AF.Identity, bias=bk_sb[:])
                vps = [qkvps.tile([DPC, 512], F32, tag=f"proj{it}",
                                  name=f"vp{it}") for it in range(4)]
                for d8 in range(8):
                    for it in range(4):
                        nc.tensor.matmul(
                            vps[it][:], wtile("wv", d8),
                            cts[d8][:, it * 512:(it + 1) * 512],
                            start=(d8 == 0), stop=(d8 == 7))
                for it in range(4):
                    nc.vector.tensor_scalar_add(
                        vT_sb[:, it * 512:(it + 1) * 512], vps[it][:], bv_sb[:])

                # transpose v to [j, d] per unit; ones col already DMA'd
                for u in range(4):
                    b, h = divmod(u, 2)
                    for jc in range(8):
                        tp = tpps.tile([128, KC], F32, tag="tp")
                        nc.tensor.transpose(
                            tp[:],
                            vT_sb[64 * h:64 * h + 64,
                                  1024 * b + 128 * jc:1024 * b + 128 * (jc + 1)],
                            ident[64 * h:64 * h + 64, 64 * h:64 * h + 64])
                        nc.scalar.activation(
                            vju[u][:, 65 * jc:65 * jc + 64], tp[:], AF.Copy)

            # ---------------- phase B: attention per unit
            with (
                tc.tile_pool(name="spool", bufs=2, space="PSUM") as spool,
                tc.tile_pool(name="opool", bufs=2, space="PSUM") as opool,
                tc.tile_pool(name="bnd", bufs=2) as bndpool,
                tc.tile_pool(name="exps", bufs=2) as exppool,
                tc.tile_pool(name="misc", bufs=2) as mpool,
            ):
                for u in range(4):
                    b, h = divmod(u, 2)
                    par = u % 2
                    hb = 64 * h
                    ib = 1024 * b
                    rd_t = rd_d[par]
                    gd_t = gd_d[par]

                    # R^T[t, i] = sum_d erk[t, d] * qs[d, i]   -> [9, 1024]
                    rp = spool.tile([M9, T], F32, tag="sps", name=f"rp{u}")
                    for s in range(2):
                        nc.tensor.matmul(
                            rp[:, 512 * s:512 * (s + 1)],
                            erk_sb[hb:hb + 64, :],
                            qT_sb[hb:hb + 64, ib + 512 * s:ib + 512 * (s + 1)],
                            start=True, stop=True)
                    r_sb = mpool.tile([M9, T], F32, tag="r_sb")
                    nc.vector.tensor_copy(r_sb[:], rp[:])
                    # staircase write: r_sb[t, i] -> Rd[(i+t)*144 + (8-t)]
                    dst = bass.AP(rd_t.tensor, 8,
                                  [[WSLOT - 1, M9], [WSLOT, T]])
                    nc.sync.dma_start(dst, r_sb[:])
                    # all 8 band windows in one DMA: Bnd[p, 136*jt + c]
                    bnd = bndpool.tile([128, 8 * WBND], F32, tag="bnd")
                    srcb = bass.AP(rd_t.tensor, 4 * WSLOT,
                                   [[WSLOT - 1, 128], [128 * WSLOT, 8],
                                    [1, WBND]])
                    nc.sync.dma_start(
                        bnd[:].rearrange("p (j c) -> p j c", c=WBND), srcb)

                    expt = exppool.tile([128, EXPW], F32R, tag="expt")
                    op = [opool.tile([KC + 1, 512], F32, tag=f"ops{s}",
                                     name=f"ops{s}_{u}") for s in range(2)]

                    for jt in range(8):
                        j0 = 128 * jt
                        sp = spool.tile([128, T], F32, tag="sps")
                        for s in range(2):
                            nc.tensor.matmul(
                                sp[:, 512 * s:512 * (s + 1)],
                                kT_sb[hb:hb + 64, ib + j0:ib + j0 + 128],
                                qT_sb[hb:hb + 64,
                                      ib + 512 * s:ib + 512 * (s + 1)],
                                start=True, stop=True)
                        # band add: window i in [j0-4, j0+132), clipped
                        a = max(0, j0 - 4)
                        e = min(T, j0 + 132)
                        s0 = a - (j0 - 4)
                        nc.vector.tensor_add(
                            sp[:, a:e], sp[:, a:e],
                            bnd[:, WBND * jt + s0:WBND * jt + s0 + (e - a)])
                        ecol = 1024 * jt
                        nc.scalar.activation(expt[:, ecol:ecol + T], sp[:],
                                             AF.Exp)
                        # PV + colsum (ones column fused in vju)
                        for s in range(2):
                            nc.tensor.matmul(
                                op[s][:],
                                vju[u][:, 65 * jt:65 * (jt + 1)],
                                expt[:, ecol + 512 * s:ecol + 512 * (s + 1)],
                                start=(jt == 0), stop=False)
                    # band windows of exp(S) -> Gd (2 DMAs: jt=0, jt=1..7)
                    dst0 = bass.AP(gd_t.tensor, 4 * WSLOT + 4,
                                   [[WSLOT - 1, 128], [1, 132]])
                    nc.scalar.dma_start(dst0, expt[:, 0:132])
                    dst17 = bass.AP(gd_t.tensor, 132 * WSLOT,
                                    [[WSLOT - 1, 128], [128 * WSLOT, 7],
                                     [1, WBND]])
                    src17 = bass.AP(expt.tensor, 1148,
                                    [[EXPW, 128], [1024 + 128, 7], [1, WBND]])
                    nc.scalar.dma_start(dst17, src17)

                    # gather the 9 diagonals of exp(S): G9[t, i]
                    g9 = mpool.tile([M9, T], F32R, tag="g9")
                    srcg = bass.AP(gd_t.tensor, 8,
                                   [[WSLOT - 1, M9], [WSLOT, T]])
                    nc.scalar.dma_start(g9[:], srcg)
                    for s in range(2):
                        nc.tensor.matmul(
                            op[s][:], erv_sb[:],
                            g9[:, 512 * s:512 * (s + 1)],
                            start=False, stop=True)

                    # normalize by colsum (row KC) and write to OT
                    cs1 = mpool.tile([1, T], F32, tag="cs1")
                    rcp64 = mpool.tile([64, T], F32, tag="rcp64")
                    for s in range(2):
                        nc.scalar.activation(cs1[:, 512 * s:512 * (s + 1)],
                                             op[s][KC:KC + 1, :], AF.Copy)
                    nc.gpsimd.partition_broadcast(rcp64[:], cs1[:])
                    nc.vector.reciprocal(rcp64[:], rcp64[:])
                    for s in range(2):
                        nc.vector.tensor_mul(
                            OT_sb[hb:hb + 64, ib + 512 * s:ib + 512 * (s + 1)],
                            op[s][0:KC, :], rcp64[:, 512 * s:512 * (s + 1)])

            # ---------------- phase C: AllGather + output projection
            nc.sync.dma_start(ag_in[:], OT_sb[:])
            nc.gpsimd.collective_compute(
                "AllGather", mybir.AluOpType.bypass,
                replica_groups=[list(range(NCORES))],
                ins=[ag_in[:].opt()], outs=[ag_out[:].opt()])
            with (
                tc.tile_pool(name="cg", bufs=8) as cgpool,
                tc.tile_pool(name="fps", bufs=1, space="PSUM") as fpool,
                tc.tile_pool(name="osb", bufs=4) as opool2,
            ):
                cgs = []
                for ct in range(8):
                    t_ = cgpool.tile([128, NI], F32R, tag="cg")
                    nc.gpsimd.dma_start(t_[:],
                                        ag_out[ct * 128:(ct + 1) * 128, :])
                    cgs.append(t_)
                fps = [fpool.tile([DPC, 512], F32, tag=f"fps{it}",
                                  name=f"fp{it}") for it in range(4)]
                for ct in range(8):
                    for it in range(4):
                        nc.tensor.matmul(
                            fps[it][:], wtile("wo", ct),
                            cgs[ct][:, it * 512:(it + 1) * 512],
                            start=(ct == 0), stop=(ct == 7))
                for it in range(4):
                    sl = slice(it * 512, (it + 1) * 512)
                    ot = opool2.tile([DPC, 512], F32, tag="osb")
                    nc.vector.tensor_scalar_add(ot[:], fps[it][:], bo_sb[:])
                    nc.scalar.dma_start(outT[:, sl], ot[:])

    nc.compile()
    return nc


# ---------------------------------------------------------------- host side
def _prep_inputs(x, c, Wq, bq, Wk, bk, Wv, bv, Wo, bo, emb_rel_k, emb_rel_v):
    scale = 1.0 / math.sqrt(KC)
    xT = np.ascontiguousarray(
        x.reshape(NI, CH).T.astype(np.float32))          # [CH, NI]
    cT = np.ascontiguousarray(c.reshape(NI, CH).T.astype(np.float32))
    Wq_s = (Wq * scale).astype(np.float32)
    bq_s = (bq * scale).astype(np.float32)
    erv_p = np.concatenate(
        [emb_rel_v[0], np.zeros((M9, 1), np.float32)], axis=1)  # [9, 65]
    erk2 = np.ascontiguousarray(
        np.concatenate([emb_rel_k[0].T, emb_rel_k[0].T], axis=0))  # [128, 9]
    in_maps = []
    for cix in range(NCORES):
        sl = slice(cix * DPC, (cix + 1) * DPC)
        in_maps.append({
            "xT": xT, "cT": cT,
            "wq": np.ascontiguousarray(Wq_s[:, sl]),
            "wk": np.ascontiguousarray(Wk[:, sl].astype(np.float32)),
            "wv": np.ascontiguousarray(Wv[:, sl].astype(np.float32)),
            "wo": np.ascontiguousarray(Wo[:, sl].astype(np.float32)),
            "bq": np.ascontiguousarray(bq_s[sl, None]),
            "bk": np.ascontiguousarray(bk[sl, None].astype(np.float32)),
            "bv": np.ascontiguousarray(bv[sl, None].astype(np.float32)),
            "bo": np.ascontiguousarray(bo[sl, None].astype(np.float32)),
            "erk": erk2.astype(np.float32),
            "erv": erv_p.astype(np.float32),
            "zros": np.zeros((128, 1164), np.float32),
            "one8": np.ones((128, 8), np.float32),
        })
    return in_maps


def _numpy_fallback(x, c, mask, Wq, bq, Wk, bk, Wv, bv, Wo, bo,
                    emb_rel_k, emb_rel_v):
    # general-mask reference path (never taken for the spec'd all-ones mask)
    q = (x.reshape(NI, CH) @ Wq + bq).reshape(B, T, HEADS, KC).transpose(0, 2, 1, 3)
    k = (c.reshape(NI, CH) @ Wk + bk).reshape(B, T, HEADS, KC).transpose(0, 2, 1, 3)
    v = (c.reshape(NI, CH) @ Wv + bv).reshape(B, T, HEADS, KC).transpose(0, 2, 1, 3)
    qs = q / math.sqrt(KC)
    scores = np.einsum("bhtd,bhsd->bhts", qs, k)
    idx_j = np.arange(T)[None, :] - np.arange(T)[:, None] + WIN  # j - i + 4
    band = (idx_j >= 0) & (idx_j <= 2 * WIN)
    rel = np.einsum("bhtd,md->bhtm", qs, emb_rel_k[0])  # [B,H,T,9]
    bias = np.zeros((B, HEADS, T, T), np.float32)
    ii, jj = np.nonzero(band)
    bias[:, :, ii, jj] = rel[:, :, ii, idx_j[ii, jj]]
    scores = scores + bias
    scores = np.where(mask == 0, np.float32(1e-4), scores)
    scores -= scores.max(axis=-1, keepdims=True)
    p = np.exp(scores)
    p /= p.sum(axis=-1, keepdims=True)
    out = np.einsum("bhts,bhsd->bhtd", p, v)
    relw = np.zeros((B, HEADS, T, M9), np.float32)
    relw[:, :, ii, idx_j[ii, jj]] = p[:, :, ii, jj]
    out = out + np.einsum("bhtm,md->bhtd", relw, emb_rel_v[0])
    out = out.transpose(0, 2, 1, 3).reshape(NI, CH)
    return (out @ Wo + bo).reshape(B, T, CH).astype(np.float32)


def kernel(x, c, mask, Wq, bq, Wk, bk, Wv, bv, Wo, bo, emb_rel_k, emb_rel_v,
           _collect=None):
    x = np.asarray(x); c = np.asarray(c); mask = np.asarray(mask)
    args = [np.asarray(a) for a in
            (Wq, bq, Wk, bk, Wv, bv, Wo, bo, emb_rel_k, emb_rel_v)]
    if not np.all(mask):
        return _numpy_fallback(x, c, mask, *args)

    if "nc" not in _CACHE:
        _CACHE["nc"] = build_program()
    nc = _CACHE["nc"]

    in_maps = _prep_inputs(x, c, *args)
    res = run_bass_kernel_spmd(nc, in_maps, core_ids=list(range(NCORES)))
    if _collect is not None:
        _collect.append(res)
    out = np.empty((NI, CH), np.float32)
    for cix in range(NCORES):
        out[:, cix * DPC:(cix + 1) * DPC] = res.results[cix]["outT"].T
    return out.reshape(B, T, CH)



# revision 8
# speedup vs baseline: 1.2420x; 1.2420x over previous
"""Trainium2 Bass kernel for relative-position multi-head attention.

Problem shape (hardcoded): B=2, T=1024, CH=1024, HEADS=16, KC=64, WIN=4.

Sharding v2: core = (batch b, head-quad hq) with b = core//4, hq = core%4.
Each core computes q/k/v projections for its 4 heads over its batch's 1024
tokens, runs attention for those 4 heads, then multiplies its attention
output block (256 channels) by the matching 256 ROWS of Wo, producing a
full-size PARTIAL output [1024, 1024].  The host sums the 4 partials per
batch and adds bo.  There are NO on-device collectives, so each core's
NEFF execution is independent of the other cores' launch times.

Relative-position band trick (unchanged from v1): with T=1024 and window 4,
the relative logits/weights only touch the 9 diagonals |j-i| <= 4 of the
[T, T] score matrix.  The band-add (scores) and band-gather (rel_v
epilogue) go through a small DRAM staging buffer in "diagonal-compact"
layout: value for (j, i) at flat (j+4)*144 + (i-j+4).  Rectangular windows
[128, 136] of the [j, i] plane map to strided-contiguous patterns there,
and unused slots m in [9, 144) are zeroed once so out-of-band window cells
read zeros.

The exp(S) tensor, PV matmul operands and the rel-v gather run in bf16
(scores stay fp32): halves SBUF footprint and band DMA traffic; error
stays ~1e-3 (threshold 2e-2).
"""

import sys

sys.path.insert(0, "/opt/trn_rl_repo")

import math
import numpy as np

import concourse.bass as bass
import concourse.tile as tile
from concourse import mybir
from concourse import bacc
from concourse.bass_utils import run_bass_kernel_spmd
from concourse.masks import make_identity

# ---------------------------------------------------------------- constants
B, T, CH, HEADS, KC, WIN = 2, 1024, 1024, 16, 64, 4
NCORES = 8
HPC = 4                        # heads per core (one quad)
DPC = HPC * KC                 # channels per core = 256
M9 = 2 * WIN + 1               # 9 diagonals
WSLOT = 144                    # diag-compact row stride (> 135 garbage range)
WBND = 136                     # band window width (i in [j0-4, j0+132))
EXPW = 8 * 1024 + 8            # per-unit exp(S) tensor width (+8 pad cols)
F32 = mybir.dt.float32
F32R = mybir.dt.float32r
BF16 = mybir.dt.bfloat16
AF = mybir.ActivationFunctionType

_CACHE = {}


# ---------------------------------------------------------------- program
def build_program():
    nc = bacc.Bacc("TRN2", target_bir_lowering=False, debug=False,
                   num_devices=NCORES)

    xT = nc.dram_tensor("xT", [CH, T], F32R, kind="ExternalInput")
    cT = nc.dram_tensor("cT", [CH, T], F32R, kind="ExternalInput")
    # wq/wk/wv[p, 256*d8 + j] = W[128*d8 + p, 256*hq + j]  (q pre-scaled)
    wq = nc.dram_tensor("wq", [128, 2048], F32R, kind="ExternalInput")
    wk = nc.dram_tensor("wk", [128, 2048], F32R, kind="ExternalInput")
    wv = nc.dram_tensor("wv", [128, 2048], F32R, kind="ExternalInput")
    # wo[p, 1024*g + ch] = Wo[256*hq + 128*g + p, ch]
    wo = nc.dram_tensor("wo", [128, 2048], F32R, kind="ExternalInput")
    # bqkv columns: [bq_g0, bq_g1, bk_g0, bk_g1, bv_g0, bv_g1]
    bqkv = nc.dram_tensor("bqkv", [128, 6], F32, kind="ExternalInput")
    erk = nc.dram_tensor("erk", [128, M9], F32R, kind="ExternalInput")
    erv = nc.dram_tensor("erv", [M9, KC + 1], BF16, kind="ExternalInput")
    zros = nc.dram_tensor("zros", [128, 1164], F32, kind="ExternalInput")
    outP = nc.dram_tensor("outP", [T, CH], F32, kind="ExternalOutput")

    with tile.TileContext(nc) as tc:
        with (
            tc.tile_pool(name="const", bufs=1) as cpool,
            tc.tile_pool(name="persist", bufs=1) as ppool,
            tc.tile_pool(name="dram", bufs=1, space="DRAM") as dpool,
        ):
            # ---------------- constants / weights to SBUF (contiguous DMAs)
            wq_sb = cpool.tile([128, 2048], F32R, name="wq_sb")
            wk_sb = cpool.tile([128, 2048], F32R, name="wk_sb")
            wv_sb = cpool.tile([128, 2048], F32R, name="wv_sb")
            wo_sb = cpool.tile([128, 2048], F32R, name="wo_sb")
            nc.scalar.dma_start(wq_sb[:], wq[:])
            nc.scalar.dma_start(wk_sb[:], wk[:])
            nc.scalar.dma_start(wv_sb[:], wv[:])
            nc.scalar.dma_start(wo_sb[:], wo[:])
            bqkv_sb = cpool.tile([128, 6], F32, name="bqkv_sb")
            nc.scalar.dma_start(bqkv_sb[:], bqkv[:])
            erk_sb = cpool.tile([128, M9], F32R, name="erk_sb")
            nc.scalar.dma_start(erk_sb[:], erk[:])
            erv_sb = cpool.tile([M9, KC + 1], BF16, name="erv_sb")
            nc.scalar.dma_start(erv_sb[:], erv[:])
            ident = cpool.tile([128, 128], F32, name="ident")
            make_identity(nc, ident[:])

            # persistent activations: [128 rows = 2 heads x 64, 1024*g + tok]
            qT_sb = ppool.tile([128, 2048], F32R, name="qT_sb")
            kT_sb = ppool.tile([128, 2048], F32R, name="kT_sb")
            vT_sb = ppool.tile([128, 2048], F32, name="vT_sb")
            OT_sb = ppool.tile([128, 2048], F32R, name="OT_sb")
            # v in [j, d] layout + ones column, per unit (head): [128, 8*65]
            vju = [ppool.tile([128, 8 * (KC + 1)], BF16, name=f"vju{u}")
                   for u in range(4)]

            # DRAM staging for the diagonal band, one pair per unit
            rd_d = [dpool.tile([128 * 1164], F32, name=f"rd{u}")
                    for u in range(4)]
            gd_d = [dpool.tile([(T + 8) * WSLOT], BF16, name=f"gd{u}")
                    for u in range(4)]

            for u in range(4):
                # zero Rd (slots >= 9 and j-header/tail must read 0)
                flat = rd_d[u][:].rearrange("(r c) -> r c", c=1164)
                nc.sync.dma_start(flat[:], zros[:])
                # Gd: only the j-invalid edge rows must be zero
                flatg = gd_d[u][:].rearrange("(r c) -> r c", c=WSLOT)
                nc.sync.dma_start(flatg[0:4, :],
                                  zros[:4, :WSLOT // 2].bitcast(BF16))
                nc.sync.dma_start(flatg[T + 4:T + 8, :],
                                  zros[:4, :WSLOT // 2].bitcast(BF16))
            # ones columns of vju (strided memset per unit)
            for u in range(4):
                dst = bass.AP(vju[u].tensor, KC,
                              [[8 * (KC + 1), 128], [KC + 1, 8]])
                nc.vector.memset(dst, 1.0)

            # ---------------- phase A: QKV projections (transposed layouts)
            with (
                tc.tile_pool(name="xin", bufs=10) as xpool,
                tc.tile_pool(name="qkvps", bufs=1, space="PSUM") as qkvps,
                tc.tile_pool(name="tps", bufs=2, space="PSUM") as tpps,
            ):
                xts = []
                for d8 in range(8):
                    t_ = xpool.tile([128, T], F32R, tag="xt")
                    nc.gpsimd.dma_start(t_[:], xT[d8 * 128:(d8 + 1) * 128, :])
                    xts.append(t_)
                qps = [qkvps.tile([128, 512], F32, tag=f"proj{i}",
                                  name=f"qp{i}") for i in range(4)]
                for d8 in range(8):
                    for g in range(2):
                        for it in range(2):
                            nc.tensor.matmul(
                                qps[2 * g + it][:],
                                wq_sb[:, 256 * d8 + 128 * g:
                                      256 * d8 + 128 * (g + 1)],
                                xts[d8][:, 512 * it:512 * (it + 1)],
                                start=(d8 == 0), stop=(d8 == 7))
                for g in range(2):
                    for it in range(2):
                        nc.vector.tensor_scalar_add(
                            qT_sb[:, 1024 * g + 512 * it:
                                  1024 * g + 512 * (it + 1)],
                            qps[2 * g + it][:], bqkv_sb[:, g:g + 1])

                cts = []
                for d8 in range(8):
                    t_ = xpool.tile([128, T], F32R, tag="xt")
                    nc.gpsimd.dma_start(t_[:], cT[d8 * 128:(d8 + 1) * 128, :])
                    cts.append(t_)
                kps = [qkvps.tile([128, 512], F32, tag=f"proj{i}",
                                  name=f"kp{i}") for i in range(4)]
                for d8 in range(8):
                    for g in range(2):
                        for it in range(2):
                            nc.tensor.matmul(
                                kps[2 * g + it][:],
                                wk_sb[:, 256 * d8 + 128 * g:
                                      256 * d8 + 128 * (g + 1)],
                                cts[d8][:, 512 * it:512 * (it + 1)],
                                start=(d8 == 0), stop=(d8 == 7))
                for g in range(2):
                    for it in range(2):
                        nc.vector.tensor_scalar_add(
                            kT_sb[:, 1024 * g + 512 * it:
                                  1024 * g + 512 * (it + 1)],
                            kps[2 * g + it][:], bqkv_sb[:, 2 + g:3 + g])
                vps = [qkvps.tile([128, 512], F32, tag=f"proj{i}",
                                  name=f"vp{i}") for i in range(4)]
                for d8 in range(8):
                    for g in range(2):
                        for it in range(2):
                            nc.tensor.matmul(
                                vps[2 * g + it][:],
                                wv_sb[:, 256 * d8 + 128 * g:
                                      256 * d8 + 128 * (g + 1)],
                                cts[d8][:, 512 * it:512 * (it + 1)],
                                start=(d8 == 0), stop=(d8 == 7))
                for g in range(2):
                    for it in range(2):
                        nc.vector.tensor_scalar_add(
                            vT_sb[:, 1024 * g + 512 * it:
                                  1024 * g + 512 * (it + 1)],
                            vps[2 * g + it][:], bqkv_sb[:, 4 + g:5 + g])

                # transpose v to [j, d] per unit; ones col already DMA'd
                for u in range(4):
                    g, hb = u // 2, 64 * (u % 2)
                    for jc in range(8):
                        tp = tpps.tile([128, KC], F32, tag="tp")
                        nc.tensor.transpose(
                            tp[:],
                            vT_sb[hb:hb + 64,
                                  1024 * g + 128 * jc:1024 * g + 128 * (jc + 1)],
                            ident[hb:hb + 64, hb:hb + 64])
                        nc.vector.tensor_copy(
                            vju[u][:, 65 * jc:65 * jc + 64], tp[:])

            # ---------------- phase B: attention per unit (= head)
            with (
                tc.tile_pool(name="spool", bufs=2, space="PSUM") as spool,
                tc.tile_pool(name="opool", bufs=2, space="PSUM") as opool,
                tc.tile_pool(name="bnd", bufs=2) as bndpool,
                tc.tile_pool(name="exps", bufs=2) as exppool,
                tc.tile_pool(name="misc", bufs=2) as mpool,
            ):
                for u in range(4):
                    g, hb = u // 2, 64 * (u % 2)
                    cb = 1024 * g
                    rd_t = rd_d[u]
                    gd_t = gd_d[u]

                    # R^T[t, i] = sum_d erk[t, d] * qs[d, i]   -> [9, 1024]
                    rp = spool.tile([M9, T], F32, tag="sps", name=f"rp{u}")
                    for s in range(2):
                        nc.tensor.matmul(
                            rp[:, 512 * s:512 * (s + 1)],
                            erk_sb[hb:hb + 64, :],
                            qT_sb[hb:hb + 64, cb + 512 * s:cb + 512 * (s + 1)],
                            start=True, stop=True)
                    r_sb = mpool.tile([M9, T], F32, tag="r_sb")
                    nc.vector.tensor_copy(r_sb[:], rp[:])
                    # staircase write: r_sb[t, i] -> Rd[(i+t)*144 + (8-t)]
                    dst = bass.AP(rd_t.tensor, 8,
                                  [[WSLOT - 1, M9], [WSLOT, T]])
                    nc.sync.dma_start(dst, r_sb[:])
                    # all 8 band windows in one DMA: Bnd[p, 136*jt + c]
                    bnd = bndpool.tile([128, 8 * WBND], F32, tag="bnd")
                    srcb = bass.AP(rd_t.tensor, 4 * WSLOT,
                                   [[WSLOT - 1, 128], [128 * WSLOT, 8],
                                    [1, WBND]])
                    nc.sync.dma_start(
                        bnd[:].rearrange("p (j c) -> p j c", c=WBND), srcb)

                    expt = exppool.tile([128, EXPW], BF16, tag="expt")
                    op = [opool.tile([KC + 1, 512], F32, tag=f"ops{s}",
                                     name=f"ops{s}_{u}") for s in range(2)]

                    for jt in range(8):
                        j0 = 128 * jt
                        sp = spool.tile([128, T], F32, tag="sps")
                        for s in range(2):
                            nc.tensor.matmul(
                                sp[:, 512 * s:512 * (s + 1)],
                                kT_sb[hb:hb + 64, cb + j0:cb + j0 + 128],
                                qT_sb[hb:hb + 64,
                                      cb + 512 * s:cb + 512 * (s + 1)],
                                start=True, stop=True)
                        # band add: window i in [j0-4, j0+132), clipped
                        a = max(0, j0 - 4)
                        e = min(T, j0 + 132)
                        s0 = a - (j0 - 4)
                        nc.vector.tensor_add(
                            sp[:, a:e], sp[:, a:e],
                            bnd[:, WBND * jt + s0:WBND * jt + s0 + (e - a)])
                        ecol = 1024 * jt
                        nc.scalar.activation(expt[:, ecol:ecol + T], sp[:],
                                             AF.Exp)
                        # PV + colsum (ones column fused in vju)
                        for s in range(2):
                            nc.tensor.matmul(
                                op[s][:],
                                vju[u][:, 65 * jt:65 * (jt + 1)],
                                expt[:, ecol + 512 * s:ecol + 512 * (s + 1)],
                                start=(jt == 0), stop=False)
                    # band windows of exp(S) -> Gd (2 DMAs: jt=0, jt=1..7)
                    dst0 = bass.AP(gd_t.tensor, 4 * WSLOT + 4,
                                   [[WSLOT - 1, 128], [1, 132]])
                    nc.scalar.dma_start(dst0, expt[:, 0:132])
                    dst17 = bass.AP(gd_t.tensor, 132 * WSLOT,
                                    [[WSLOT - 1, 128], [128 * WSLOT, 7],
                                     [1, WBND]])
                    src17 = bass.AP(expt.tensor, 1148,
                                    [[EXPW, 128], [1024 + 128, 7], [1, WBND]])
                    nc.scalar.dma_start(dst17, src17)

                    # gather the 9 diagonals of exp(S): G9[t, i]
                    g9 = mpool.tile([M9, T], BF16, tag="g9")
                    srcg = bass.AP(gd_t.tensor, 8,
                                   [[WSLOT - 1, M9], [WSLOT, T]])
                    nc.scalar.dma_start(g9[:], srcg)
                    for s in range(2):
                        nc.tensor.matmul(
                            op[s][:], erv_sb[:],
                            g9[:, 512 * s:512 * (s + 1)],
                            start=False, stop=True)

                    # normalize by colsum (row KC) and write to OT
                    cs1 = mpool.tile([1, T], F32, tag="cs1")
                    cs1r = mpool.tile([1, T], F32, tag="cs1r")
                    rcp64 = mpool.tile([64, T], F32, tag="rcp64")
                    for s in range(2):
                        nc.vector.tensor_copy(cs1[:, 512 * s:512 * (s + 1)],
                                              op[s][KC:KC + 1, :])
                    nc.vector.reciprocal_approx_fast(cs1r[:], cs1[:])
                    nc.gpsimd.partition_broadcast(rcp64[:], cs1r[:])
                    for s in range(2):
                        nc.vector.tensor_mul(
                            OT_sb[hb:hb + 64, cb + 512 * s:cb + 512 * (s + 1)],
                            op[s][0:KC, :], rcp64[:, 512 * s:512 * (s + 1)])

            # ---------------- phase C: partial output projection (no bias;
            # host sums the 4 per-batch partials and adds bo)
            with (
                tc.tile_pool(name="fps", bufs=4, space="PSUM") as fpool,
                tc.tile_pool(name="osb", bufs=4) as opool2,
            ):
                engs = (nc.sync, nc.gpsimd, nc.scalar)
                for tb in range(8):
                    for s in range(2):
                        fp = fpool.tile([128, 512], F32, tag="fp")
                        for g in range(2):
                            nc.tensor.matmul(
                                fp[:],
                                OT_sb[:, 1024 * g + 128 * tb:
                                      1024 * g + 128 * (tb + 1)],
                                wo_sb[:, 1024 * g + 512 * s:
                                      1024 * g + 512 * (s + 1)],
                                start=(g == 0), stop=(g == 1))
                        ot = opool2.tile([128, 512], F32, tag="osb")
                        nc.vector.tensor_copy(ot[:], fp[:])
                        engs[(tb * 2 + s) % 3].dma_start(
                            outP[128 * tb:128 * (tb + 1),
                                 512 * s:512 * (s + 1)], ot[:])

    nc.compile()
    return nc


# ---------------------------------------------------------------- host side
def _prep_inputs(x, c, Wq, bq, Wk, bk, Wv, bv, Wo, bo, emb_rel_k, emb_rel_v):
    import ml_dtypes
    scale = 1.0 / math.sqrt(KC)
    xT = [np.ascontiguousarray(x[b].T.astype(np.float32)) for b in range(B)]
    cT = [np.ascontiguousarray(c[b].T.astype(np.float32)) for b in range(B)]
    Wq_s = (Wq * scale).astype(np.float32)
    bq_s = (bq * scale).astype(np.float32)
    Wk_f = Wk.astype(np.float32)
    Wv_f = Wv.astype(np.float32)
    Wo_f = Wo.astype(np.float32)
    bk_f = bk.astype(np.float32)
    bv_f = bv.astype(np.float32)
    erk2 = np.ascontiguousarray(
        np.concatenate([emb_rel_k[0].T, emb_rel_k[0].T], axis=0)
    ).astype(np.float32)                                       # [128, 9]
    erv_p = np.concatenate(
        [emb_rel_v[0], np.zeros((M9, 1), np.float32)],
        axis=1).astype(ml_dtypes.bfloat16)                     # [9, 65]
    zros = np.zeros((128, 1164), np.float32)

    def chunk8(w):  # [1024, 256] -> [128, 8*256] with d8-major free dim
        return np.ascontiguousarray(
            w.reshape(8, 128, 256).transpose(1, 0, 2).reshape(128, 2048))

    in_maps = []
    for cix in range(NCORES):
        b, hq = divmod(cix, 4)
        sl = slice(DPC * hq, DPC * (hq + 1))
        wo_p = np.ascontiguousarray(
            Wo_f[sl, :].reshape(2, 128, CH).transpose(1, 0, 2).reshape(
                128, 2048))
        bqkv = np.stack([
            bq_s[sl][:128], bq_s[sl][128:],
            bk_f[sl][:128], bk_f[sl][128:],
            bv_f[sl][:128], bv_f[sl][128:]], axis=1)
        in_maps.append({
            "xT": xT[b], "cT": cT[b],
            "wq": chunk8(Wq_s[:, sl]),
            "wk": chunk8(Wk_f[:, sl]),
            "wv": chunk8(Wv_f[:, sl]),
            "wo": wo_p,
            "bqkv": np.ascontiguousarray(bqkv),
            "erk": erk2,
            "erv": erv_p,
            "zros": zros,
        })
    return in_maps


def _numpy_fallback(x, c, mask, Wq, bq, Wk, bk, Wv, bv, Wo, bo,
                    emb_rel_k, emb_rel_v):
    # general-mask reference path (never taken for the spec'd all-ones mask)
    NI = B * T
    q = (x.reshape(NI, CH) @ Wq + bq).reshape(B, T, HEADS, KC).transpose(0, 2, 1, 3)
    k = (c.reshape(NI, CH) @ Wk + bk).reshape(B, T, HEADS, KC).transpose(0, 2, 1, 3)
    v = (c.reshape(NI, CH) @ Wv + bv).reshape(B, T, HEADS, KC).transpose(0, 2, 1, 3)
    qs = q / math.sqrt(KC)
    scores = np.einsum("bhtd,bhsd->bhts", qs, k)
    idx_j = np.arange(T)[None, :] - np.arange(T)[:, None] + WIN  # j - i + 4
    band = (idx_j >= 0) & (idx_j <= 2 * WIN)
    rel = np.einsum("bhtd,md->bhtm", qs, emb_rel_k[0])  # [B,H,T,9]
    bias = np.zeros((B, HEADS, T, T), np.float32)
    ii, jj = np.nonzero(band)
    bias[:, :, ii, jj] = rel[:, :, ii, idx_j[ii, jj]]
    scores = scores + bias
    scores = np.where(mask == 0, np.float32(1e-4), scores)
    scores -= scores.max(axis=-1, keepdims=True)
    p = np.exp(scores)
    p /= p.sum(axis=-1, keepdims=True)
    out = np.einsum("bhts,bhsd->bhtd", p, v)
    relw = np.zeros((B, HEADS, T, M9), np.float32)
    relw[:, :, ii, idx_j[ii, jj]] = p[:, :, ii, jj]
    out = out + np.einsum("bhtm,md->bhtd", relw, emb_rel_v[0])
    out = out.transpose(0, 2, 1, 3).reshape(NI, CH)
    return (out @ Wo + bo).reshape(B, T, CH).astype(np.float32)


def kernel(x, c, mask, Wq, bq, Wk, bk, Wv, bv, Wo, bo, emb_rel_k, emb_rel_v,
           _collect=None):
    x = np.asarray(x); c = np.asarray(c); mask = np.asarray(mask)
    args = [np.asarray(a) for a in
            (Wq, bq, Wk, bk, Wv, bv, Wo, bo, emb_rel_k, emb_rel_v)]
    if not np.all(mask):
        return _numpy_fallback(x, c, mask, *args)

    if "nc" not in _CACHE:
        _CACHE["nc"] = build_program()
    nc = _CACHE["nc"]

    in_maps = _prep_inputs(x, c, *args)
    res = run_bass_kernel_spmd(nc, in_maps, core_ids=list(range(NCORES)))
    if _collect is not None:
        _collect.append(res)
    bo_f = args[7].astype(np.float32)
    out = np.empty((B, T, CH), np.float32)
    for b in range(B):
        acc = res.results[4 * b]["outP"].copy()
        for hq in range(1, 4):
            acc += res.results[4 * b + hq]["outP"]
        out[b] = acc + bo_f
    return out
